# revision 20
# baseline (speedup 1.0000x reference)
"""Trainium2 Bass kernel for nn_AudioMixer (4-track stereo mixer:
per-track 3-stage biquad EQ -> compressor -> Schroeder reverb on tracks 2,3
-> pan/volume mix -> limiter clip).

Sharding: core c = (track c//2, channel c%2) — each of the 8 cores processes
one full (track, channel) row of 1.44M samples end-to-end, then a
ReduceScatter over channel groups {0,2,4,6} / {1,3,5,7} sums the 4 weighted
tracks per channel; each core clips + writes a quarter of its channel.

Algorithms (validated in numpy against the jax reference):
 - EQ: combined per-track state-space cascade (order 6). Per 128-sample block:
   zero-state response via a lower-triangular Toeplitz matmul on the PE;
   cross-block state corrections via two more matmuls whose rhs are stacked
   shifted "tail" rows (last 6 x / y0 rows per block). Exact to f32 rounding.
 - Compressor: attack/release envelope via policy iteration: branch pattern m
   from the previous iterate, then env = c*env + (1-c)*lvl runs on the
   hardware tensor_tensor_scan (128 lanes x 11264). it0 (all-release) gets an
   exact cross-lane chain fix via REL-power decay; later iterations chain
   lazily via previous-iterate lane finals. 5 iterations -> ~1.5e-4.
 - Reverb: comb y[n] = x[n] + fb*y[n-d] over "epochs" of d samples is a
   lower-triangular matmul over 128 stacked epochs (fb^(p-q)); tiles of 128
   epochs overlap by 40 warmup epochs (fb^40 ~ 6e-8) so tiles are
   independent. Same for both allpasses (the reference's first-epoch-zero
   quirk gets its own matrix for tile 0). Relayouts go through DRAM scratch.
"""
import math
from contextlib import ExitStack

import numpy as np

import concourse.bass as bass
import concourse.bacc as bacc
import concourse.mybir as mybir
import concourse.tile as tile
from concourse.bass_utils import run_bass_kernel_spmd

F32 = mybir.dt.float32
BF16 = mybir.dt.bfloat16

# ---------------------------------------------------------------- constants
SR = 48000
N = 1_440_000
NP = 128 * 11264          # padded row length (1441792)
F = 11264                 # per-lane length (128 lanes)
FCB = 2816                # EQ free-chunk (4 chunks)
FC = 1408                 # compressor free-chunk (8 chunks)
CH = NP // 4              # ReduceScatter chunk per core = 360448

ATK = math.exp(-1.0 / (10.0 * 0.001 * SR))
REL = math.exp(-1.0 / (100.0 * 0.001 * SR))
THR = 10.0 ** (-18.0 / 20.0)
GR_EXP = 1.0 / 4.0 - 1.0
_BASE = int(SR * 0.03)
COMB_DELAYS = [_BASE, int(_BASE * 1.13), int(_BASE * 1.27), int(_BASE * 1.41)]
AP_DELAYS = [int(SR * 0.005), int(SR * 0.0017)]
FB = 0.3 + 0.5 * 0.6
WET = 0.3
CEIL = 10.0 ** (-1.0 / 20.0)

N_ITER = 5                # compressor policy iterations (lazy chaining)
TAILD = 6                 # tail rows for EQ correction
JMAX = 20                 # correction shift terms (zero-padded per track)
VEP = 104               # valid epochs per reverb tile
WEP = 24                # warmup epochs per tile (fb^24 ~ 4.6e-5)

# ---------------------------------------------------------------- EQ host math
def _peak_coefs(freq, gain_db, q):
    A = 10.0 ** (gain_db / 40.0)
    w0 = 2.0 * math.pi * freq / SR
    al = math.sin(w0) / (2.0 * q)
    a0 = 1.0 + al / A
    return ((1.0 + al * A) / a0, -2.0 * math.cos(w0) / a0, (1.0 - al * A) / a0,
            -2.0 * math.cos(w0) / a0, (1.0 - al / A) / a0)

_IDENT = (1.0, 0.0, 0.0, 0.0, 0.0)
_PRESETS = {
    0: [(300.0, -3.0, 0.7), (3000.0, 3.0, 1.0), (8000.0, 2.0, 0.7)],
    1: [(80.0, 2.0, 0.7), (5000.0, 1.0, 1.0)],
    2: [(200.0, -2.0, 0.7), (6000.0, -1.0, 0.7)],
    3: [(1000.0, 2.0, 1.0)],
}

def _stage_coefs(track):
    bands = [_peak_coefs(*b) for b in _PRESETS[track]]
    bands += [_IDENT] * (3 - len(bands))
    return bands

def _biquad_ss(c):
    b0, b1, b2, a1, a2 = [float(v) for v in c]
    A = np.array([[-a1, 1.0], [-a2, 0.0]])
    B = np.array([[b1 - a1 * b0], [b2 - a2 * b0]])
    C = np.array([[1.0, 0.0]])
    D = np.array([[b0]])
    return A, B, C, D

def _cascade(ss_list):
    A1, B1, C1, D1 = ss_list[0]
    for A2, B2, C2, D2 in ss_list[1:]:
        n1, n2 = A1.shape[0], A2.shape[0]
        A = np.zeros((n1 + n2, n1 + n2))
        A[:n1, :n1] = A1
        A[n1:, :n1] = B2 @ C1
        A[n1:, n1:] = A2
        B = np.vstack([B1, B2 @ D1])
        C = np.hstack([D2 @ C1, C2])
        D = D2 @ D1
        A1, B1, C1, D1 = A, B, C, D
    return A1, B1, C1, D1

def _track_eq_consts(track, L=128):
    A, B, C, D = _cascade([_biquad_ss(c) for c in _stage_coefs(track)])
    n = A.shape[0]          # 6
    h = np.zeros(L)
    h[0] = D[0, 0]
    Ak = np.eye(n)
    for k in range(1, L):
        h[k] = (C @ Ak @ B)[0, 0]
        Ak = A @ Ak
    T = np.zeros((L, L))
    for i in range(L):
        T[i, : i + 1] = h[i::-1]
    Phi = np.zeros((L, n))
    Ak = np.eye(n)
    for k in range(L):
        Phi[k] = (C @ Ak)[0]
        Ak = A @ Ak
    A_L = Ak
    # probe-fit: s_end = Wx @ x_tail + Wy @ y_tail
    rng = np.random.default_rng(0)
    P = 64
    X = rng.standard_normal((P, L))
    feats = np.zeros((P, 2 * TAILD))
    targ = np.zeros((P, n))
    for p in range(P):
        s = np.zeros(n)
        y = np.zeros(L)
        for k in range(L):
            y[k] = (C @ s)[0] + D[0, 0] * X[p, k]
            s = A @ s + B[:, 0] * X[p, k]
        feats[p, :TAILD] = X[p, L - TAILD:]
        feats[p, TAILD:] = y[L - TAILD:]
        targ[p] = s
    Wfit = np.linalg.lstsq(feats, targ, rcond=None)[0]
    assert np.abs(feats @ Wfit - targ).max() < 1e-7
    Wx = Wfit[:TAILD].T
    Wy = Wfit[TAILD:].T
    lhsT_a = np.zeros((n * JMAX, L))
    lhsT_b = np.zeros((n * JMAX, L))
    lam = max(abs(np.linalg.eigvals(A_L)))
    J = int(np.clip(np.ceil(np.log(1e-9) / np.log(max(lam, 1e-12))), 2, JMAX))
    Ai = np.eye(n)
    for i in range(J):
        GWx = Phi @ Ai @ Wx
        GWy = Phi @ Ai @ Wy
        # device stack row (i', c) holds tails shifted by (JMAX - i') blocks,
        # i.e. shift index i = JMAX - 1 - i' -> store at reversed i slot
        ip = JMAX - 1 - i
        for c in range(TAILD):
            lhsT_a[n * ip + c] = GWx[:, c]
            lhsT_b[n * ip + c] = GWy[:, c]
        Ai = A_L @ Ai
    return T, lhsT_a, lhsT_b

# ---------------------------------------------------------------- reverb host math
def _epoch_matrix_comb(fb, L=128):
    Lm = np.zeros((L, L))
    for q in range(L):
        y = np.zeros(L)
        prev = 0.0
        for p_ in range(L):
            y[p_] = (1.0 if p_ == q else 0.0) + fb * prev
            prev = y[p_]
        Lm[:, q] = y
    return Lm

def _epoch_matrix_ap(fb, L=128, quirk=False):
    Lm = np.zeros((L, L))
    for q in range(L):
        X = np.zeros(L)
        X[q] = 1.0
        y = np.zeros(L)
        yprev = 0.0
        xprev = 0.0
        for p_ in range(L):
            y[p_] = 0.0 if (quirk and p_ == 0) else (-fb * X[p_] + xprev + fb * yprev)
            yprev = y[p_]
            xprev = X[p_]
        Lm[:, q] = y
    return Lm

def _rev_tiles(d):
    M = -(-NP // d)
    T = -(-M // VEP)
    return T, T * VEP * d   # tile count, flat coverage

_COMB_COVER = max(_rev_tiles(d)[1] for d in COMB_DELAYS)
_AP0_COVER = _rev_tiles(AP_DELAYS[0])[1]
_AP1_COVER = _rev_tiles(AP_DELAYS[1])[1]
# ap81 reads apdram up to its own tile-grid coverage; ap240 only writes its
# grid's coverage -> size apdram to the max and zero the gap
_AP0_SIZE = max(_AP0_COVER, _AP1_COVER)


# ============================================================== device program
def build_program(with_collective=True):
    nc = bacc.Bacc("TRN2", target_bir_lowering=False, debug=False)
    dt = F32
    ao = mybir.AluOpType
    AF = mybir.ActivationFunctionType

    x = nc.declare_dram_parameter("x", [NP], dt, isOutput=False)
    thT = nc.declare_dram_parameter("thT", [128, 128], dt, isOutput=False)
    ca = nc.declare_dram_parameter("ca", [TAILD * JMAX, 128], dt, isOutput=False)
    cb = nc.declare_dram_parameter("cb", [TAILD * JMAX, 128], dt, isOutput=False)
    identp = nc.declare_dram_parameter("ident", [128, 128], dt, isOutput=False)
    lcT = nc.declare_dram_parameter("lcT", [128, 128], dt, isOutput=False)
    laT = nc.declare_dram_parameter("laT", [128, 128], dt, isOutput=False)
    laqT = nc.declare_dram_parameter("laqT", [128, 128], dt, isOutput=False)
    relpow = nc.declare_dram_parameter("relpow", [128, FC], dt, isOutput=False)
    wdry = nc.declare_dram_parameter("wdry", [128, 1], dt, isOutput=False)
    wwet = nc.declare_dram_parameter("wwet", [128, 1], dt, isOutput=False)
    out = nc.declare_dram_parameter("out", [CH], dt, isOutput=True)

    ydram = nc.dram_tensor("ydram", [_COMB_COVER], dt)
    wetdram = nc.dram_tensor("wetdram", [_COMB_COVER], dt)
    apdram = nc.dram_tensor("apdram", [_AP0_SIZE], dt)
    wet2dram = nc.dram_tensor("wet2dram", [_AP1_COVER], dt)
    mixdram = nc.dram_tensor("mixdram", [NP], dt)
    xtaildram = nc.dram_tensor("xtaildram", [TAILD, 32 + F], dt)
    ytaildram = nc.dram_tensor("ytaildram", [TAILD, 32 + F], dt)
    rsdram = nc.dram_tensor("rsdram", [CH], dt)

    with tile.TileContext(nc) as tc, ExitStack() as ctx:
        cons = ctx.enter_context(tc.tile_pool(name="cons", bufs=1))
        bigs = ctx.enter_context(tc.tile_pool(name="bigs", bufs=3))
        ps = ctx.enter_context(tc.tile_pool(name="ps", bufs=4, space="PSUM"))
        tiny = ctx.enter_context(tc.tile_pool(name="tiny", bufs=2))

        # ---- constants to SBUF
        t_thT = cons.tile([128, 128], dt, tag="thT")
        t_ca = cons.tile([TAILD * JMAX, 128], dt, tag="ca")
        t_cb = cons.tile([TAILD * JMAX, 128], dt, tag="cb")
        t_id = cons.tile([128, 128], dt, tag="ident")
        t_lcT = cons.tile([128, 128], dt, tag="lcT")
        t_laT = cons.tile([128, 128], dt, tag="laT")
        t_laqT = cons.tile([128, 128], dt, tag="laqT")
        t_relpow = cons.tile([128, FC], dt, tag="relpow")
        t_wdry = cons.tile([128, 1], dt, tag="wdry")
        t_wwet = cons.tile([128, 1], dt, tag="wwet")
        t_ones = cons.tile([1, 1], dt, tag="ones")
        t_zcol = cons.tile([128, 1], dt, tag="zcol")
        for t_, src in ((t_thT, thT), (t_ca, ca), (t_cb, cb), (t_id, identp),
                        (t_lcT, lcT), (t_laT, laT), (t_laqT, laqT),
                        (t_relpow, relpow), (t_wdry, wdry), (t_wwet, wwet)):
            nc.sync.dma_start(t_[:], src[:])
        nc.gpsimd.memset(t_ones[:], 1.0)
        nc.gpsimd.memset(t_zcol[:], 0.0)

        # ================= Phase A: load x -> U tiles -> PE transpose -> xL1
        xL1 = bigs.tile([128, F], dt, tag="big")
        x4 = x[:].rearrange("(w a b) -> w a b", a=128, b=128)  # [88,128,128]
        with tc.tile_pool(name="stg", bufs=3) as stg:
            for wq8 in range(11):
                s = stg.tile([128, 8, 128], dt, tag="ustg")
                nc.sync.dma_start(
                    s[:], x4[8 * wq8: 8 * wq8 + 8].rearrange("w a b -> a w b"))
                for half in range(2):
                    wq = 2 * wq8 + half
                    pt = ps.tile([128, 512], dt, tag="pstrans")
                    for wl in range(4):
                        nc.tensor.transpose(
                            pt[:, 128 * wl: 128 * wl + 128],
                            s[:, 4 * half + wl, :], t_id[:])
                    nc.scalar.copy(xL1[:, 512 * wq: 512 * wq + 512], pt[:])

        # ================= Phase B: EQ matmuls
        y0 = bigs.tile([128, F], dt, tag="big")
        ytr = bigs.tile([128, F], dt, tag="big")
        SUBS = [512] * 5 + [256]   # 2816
        with tc.tile_pool(name="stk", bufs=2) as stkp:
            # zero pad columns + full x tails to DRAM once
            zpad = stkp.tile([TAILD, 32], dt, tag="zpad")
            nc.gpsimd.memset(zpad[:], 0.0)
            nc.sync.dma_start(xtaildram[:, 0:32], zpad[:])
            nc.sync.dma_start(ytaildram[:, 0:32], zpad[:])
            nc.sync.dma_start(xtaildram[:, 32:], xL1[122:128, :])
            for k in range(4):
                base = FCB * k
                off = 0
                for sub in SUBS:
                    p1 = ps.tile([128, 512], dt, tag="psmm")
                    nc.tensor.matmul(p1[:, :sub], t_thT[:],
                                     xL1[:, base + off: base + off + sub])
                    nc.scalar.copy(y0[:, base + off: base + off + sub],
                                   p1[:, :sub])
                    off += sub
                # this chunk's y tails to DRAM (sliding windows read them back)
                nc.sync.dma_start(ytaildram[:, 32 + base: 32 + base + FCB],
                                  y0[122:128, base: base + FCB])
                sa = stkp.tile([TAILD * JMAX, FCB], dt, tag="stack")
                sb = stkp.tile([TAILD * JMAX, FCB], dt, tag="stack")
                # single sliding-window DMA per stack: stack row (i', c) =
                # tail row c shifted by (JMAX - i') blocks (lhsT rows match)
                soff = 32 + base - JMAX
                sap = [[1, JMAX], [32 + F, TAILD], [1, FCB]]
                nc.sync.dma_start(
                    sa[:], bass.AP(tensor=xtaildram, offset=soff, ap=sap))
                nc.sync.dma_start(
                    sb[:], bass.AP(tensor=ytaildram, offset=soff, ap=sap))
                off = 0
                for sub in SUBS:
                    p2 = ps.tile([128, 512], dt, tag="psmm")
                    nc.tensor.matmul(p2[:, :sub], t_ca[:], sa[:, off: off + sub],
                                     start=True, stop=False)
                    nc.tensor.matmul(p2[:, :sub], t_cb[:], sb[:, off: off + sub],
                                     start=False, stop=True)
                    nc.vector.tensor_add(ytr[:, base + off: base + off + sub],
                                         y0[:, base + off: base + off + sub],
                                         p2[:, :sub])
                    off += sub

        # ================= Phase C: transpose back + flatten to L2 lanes
        ustg2 = bigs.tile([128, F], dt, tag="big")   # reuses xL1's slot region
        for wq in range(22):
            pt = ps.tile([128, 512], dt, tag="pstrans")
            for wl in range(4):
                w = 4 * wq + wl
                nc.tensor.transpose(pt[:, 128 * wl: 128 * wl + 128],
                                    ytr[:, 128 * w: 128 * w + 128], t_id[:])
            nc.scalar.copy(ustg2[:, 512 * wq: 512 * wq + 512], pt[:])
        yeq = bigs.tile([128, F], dt, tag="big")     # reuses y0's slot region
        # flatten via DRAM scratch (mixdram is free here): tile-major -> flat
        u3 = ustg2[:].rearrange("a (w b) -> a w b", b=128)
        md = mixdram[:].rearrange("(w a b) -> a w b", a=128, b=128)
        nc.sync.dma_start(md, u3)
        nc.sync.dma_start(yeq[:], mixdram[:].rearrange("(p f) -> p f", p=128))

        # ================= Phase D: compressor
        lvl = bigs.tile([128, F], dt, tag="big")     # reuses ytr's slot
        nc.scalar.activation(lvl[:], yeq[:], AF.Abs)
        env = bigs.tile([128, F], dt, tag="big")     # reuses ustg2's slot
        with tc.tile_pool(name="relcp", bufs=1) as relcp, \
             tc.tile_pool(name="chk", bufs=2) as chk:
            relc = relcp.tile([128, FC], dt, tag="relc")
            nc.gpsimd.memset(relc[:], REL)
            rowsc = tiny.tile([1, 130], dt, tag="rowsc")
            irow = tiny.tile([1, 128], dt, tag="irow")
            nc.gpsimd.memset(rowsc[:], 0.0)
            relFrow = tiny.tile([1, 128], dt, tag="relF")
            nc.gpsimd.memset(relFrow[:], float(REL ** F))
            icol = tiny.tile([128, 1], dt, tag="icol")
            lanecol = tiny.tile([128, 1], dt, tag="lanecol")
            savec = tiny.tile([128, 8], dt, tag="savec")

            # it0: all-release, exact chain fix
            for k in range(8):
                d1 = chk.tile([128, FC], dt, tag="d1")
                nc.scalar.mul(d1[:], lvl[:, FC * k: FC * (k + 1)], 1.0 - REL)
                init = 0.0 if k == 0 else env[:, FC * k - 1: FC * k]
                nc.vector.tensor_tensor_scan(
                    env[:, FC * k: FC * (k + 1)], relc[:], d1[:], init,
                    op0=ao.mult, op1=ao.add)
            pr = ps.tile([128, 512], dt, tag="psmm")
            nc.tensor.matmul(pr[:1, :128], env[:, F - 1: F], t_id[:])
            nc.scalar.copy(rowsc[0:1, 1:129], pr[:1, :128])
            nc.vector.tensor_tensor_scan(
                irow[:], relFrow[:], rowsc[0:1, 0:128], 0.0,
                op0=ao.mult, op1=ao.add)
            pc = ps.tile([128, 512], dt, tag="psmm")
            nc.tensor.matmul(pc[:128, :1], irow[:], t_ones[:])
            nc.scalar.copy(icol[:], pc[:128, :1])
            for k in range(8):
                isc = tiny.tile([128, 1], dt, tag="isc")
                nc.vector.tensor_scalar_mul(isc[:], icol[:],
                                            float(REL ** (FC * k)))
                nc.vector.scalar_tensor_tensor(
                    env[:, FC * k: FC * (k + 1)], t_relpow[:], isc[:, 0:1],
                    env[:, FC * k: FC * (k + 1)], op0=ao.mult, op1=ao.add)

            # lazy policy iterations
            for it in range(N_ITER):
                nc.sync.dma_start(lanecol[1:128, :], env[0:127, F - 1: F])
                nc.vector.tensor_copy(lanecol[0:1, :], t_zcol[0:1, :])
                for k in range(8):
                    nc.vector.tensor_copy(savec[:, k: k + 1],
                                          env[:, FC * (k + 1) - 1: FC * (k + 1)])
                for k in range(8):
                    base = FC * k
                    m = chk.tile([128, FC], dt, tag="mtile")
                    bc = lanecol[:, 0:1] if k == 0 else savec[:, k - 1: k]
                    nc.vector.tensor_tensor(
                        m[:, 1:], lvl[:, base + 1: base + FC],
                        env[:, base: base + FC - 1], op=ao.is_gt)
                    nc.vector.tensor_tensor(
                        m[:, 0:1], lvl[:, base: base + 1], bc, op=ao.is_gt)
                    ct = chk.tile([128, FC], dt, tag="d1")
                    nc.scalar.activation(
                        ct[:], m[:], AF.Copy,
                        bias=float(1.0 - REL),
                        scale=float((1.0 - ATK) - (1.0 - REL)))
                    nc.vector.tensor_mul(ct[:], ct[:], lvl[:, base: base + FC])
                    nc.scalar.activation(m[:], m[:], AF.Copy,
                                         bias=float(REL), scale=float(ATK - REL))
                    init = lanecol[:, 0:1] if k == 0 else env[:, base - 1: base]
                    nc.vector.tensor_tensor_scan(
                        env[:, base: base + FC], m[:], ct[:], init,
                        op0=ao.mult, op1=ao.add)

            # gr and y = x * gr (in-place onto yeq)
            for k in range(8):
                base = FC * k
                g = chk.tile([128, FC], dt, tag="mtile")
                nc.scalar.activation(g[:], env[:, base: base + FC], AF.Ln,
                                     bias=0.0, scale=float(1.0 / (THR + 1e-8)))
                nc.scalar.activation(g[:], g[:], AF.Relu)
                nc.scalar.activation(g[:], g[:], AF.Exp,
                                     bias=0.0, scale=float(GR_EXP))
                nc.vector.tensor_mul(yeq[:, base: base + FC],
                                     yeq[:, base: base + FC], g[:])
        ycomp = yeq

        # ================= Phase E: reverb (all cores; blended by wwet)
        with tc.tile_pool(name="rvin", bufs=3) as rvin, \
             tc.tile_pool(name="rvout", bufs=3) as rvout:
            nc.sync.dma_start(ydram[0:NP].rearrange("(p f) -> p f", p=128),
                              ycomp[:])
            zt = rvout.tile([128, 2048], dt, tag="rv_out")
            nc.gpsimd.memset(zt[:], 0.0)
            tail = _COMB_COVER - NP
            tf = tail // 2048
            nc.sync.dma_start(
                ydram[NP: NP + tf * 2048].rearrange("(o f) -> o f", o=tf),
                zt[0:tf, :])
            rem = tail - tf * 2048
            if rem:
                nc.sync.dma_start(
                    ydram[NP + tf * 2048:].rearrange("(o f) -> o f", o=1),
                    zt[tf: tf + 1, 0:rem])

            def epoch_filter(src_dram, dst_dram, d, lhsT_t0, lhsT, accum,
                             G=1, in_dt=dt):
                in_eng = nc.gpsimd if in_dt != dt else nc.sync
                Tt, cover = _rev_tiles(d)
                packmm = max(1, 512 // d)   # tiles per matmul (pack*d <= 512)

                def do_group(t, g):
                    # tiles t..t+g-1 (t>=1: warmup rows; t==0 solo, no warmup)
                    it_ = rvin.tile([128, g, d], in_dt, tag="rv_in")
                    if t == 0:
                        in_eng.dma_start(
                            it_[:, 0, :],
                            src_dram[0: 128 * d].rearrange("(e i) -> e i",
                                                           e=128))
                    else:
                        soff = (VEP * t - WEP) * d
                        in_eng.dma_start(
                            it_[:], bass.AP(tensor=src_dram, offset=soff,
                                            ap=[[d, 128], [VEP * d, g],
                                                [1, d]]))
                    ot = rvout.tile([128, g, d], dt, tag="rv_out")
                    lt = lhsT_t0 if t == 0 else lhsT
                    j = 0
                    while j < g:
                        pk = min(packmm, g - j)
                        sub = pk * d
                        offd = 0
                        while offd < d * pk:   # chunk if d > 512
                            s2 = min(512, sub - offd)
                            pe = ps.tile([128, 512], dt, tag="psmm")
                            rhs = (it_[:, j, offd: offd + s2] if pk == 1
                                   else it_[:, j: j + pk, :])
                            o2 = (ot[:, j, offd: offd + s2] if pk == 1
                                  else ot[:, j: j + pk, :])
                            ev = nc.scalar.copy if (t % 2 == 0) else \
                                nc.vector.tensor_copy
                            if pk == 1:
                                nc.tensor.matmul(pe[:, :s2], lt[:], rhs)
                                ev(o2, pe[:, :s2])
                            else:
                                nc.tensor.matmul(
                                    pe[:, :sub].rearrange(
                                        "p (j i) -> p j i", j=pk), lt[:], rhs)
                                ev(o2, pe[:, :sub].rearrange(
                                    "p (j i) -> p j i", j=pk))
                            offd += s2
                        j += pk
                    if t == 0:
                        rows = ot[0:VEP, 0, :]
                        dst = dst_dram[0: VEP * d].rearrange("(e i) -> e i",
                                                             e=VEP)
                        eng = nc.gpsimd if accum else nc.sync
                        eng.dma_start(dst, rows,
                                      **({"accum_op": ao.add} if accum else {}))
                    else:
                        rows = ot[WEP:128, :, :]
                        dap = bass.AP(tensor=dst_dram, offset=VEP * t * d,
                                      ap=[[d, VEP], [VEP * d, g], [1, d]])
                        eng = nc.gpsimd if accum else nc.sync
                        eng.dma_start(dap, rows,
                                      **({"accum_op": ao.add} if accum else {}))

                do_group(0, 1)
                t = 1
                while t < Tt:
                    g = min(G, Tt - t)
                    do_group(t, g)
                    t += g

            combs = sorted(COMB_DELAYS, key=lambda d: -_rev_tiles(d)[1])
            for ci, d in enumerate(combs):
                epoch_filter(ydram, wetdram, d, t_lcT, t_lcT, accum=(ci > 0))
            epoch_filter(wetdram, apdram, AP_DELAYS[0], t_laqT, t_laT, False,
                         G=8)
            if _AP0_SIZE > _AP0_COVER:
                gap = _AP0_SIZE - _AP0_COVER
                ztg = rvout.tile([128, 2048], dt, tag="rv_out")
                nc.gpsimd.memset(ztg[:], 0.0)
                nc.sync.dma_start(
                    apdram[_AP0_COVER:].rearrange("(o f) -> o f", o=1),
                    ztg[0:1, 0:gap])
            epoch_filter(apdram, wet2dram, AP_DELAYS[1], t_laqT, t_laT, False,
                         G=24)

            # ============= Phase F: mix + collective + clip
            wetl2 = bigs.tile([128, F], dt, tag="big")
            nc.sync.dma_start(wetl2[:],
                              wet2dram[0:NP].rearrange("(p f) -> p f", p=128))
            nc.scalar.activation(ycomp[:], ycomp[:], AF.Copy,
                                 scale=t_wdry[:, 0:1])
            nc.vector.scalar_tensor_tensor(
                wetl2[:], wetl2[:], t_wwet[:, 0:1], ycomp[:],
                op0=ao.mult, op1=ao.add)
            nc.sync.dma_start(mixdram[:].rearrange("(p f) -> p f", p=128),
                              wetl2[:])
            if with_collective:
                nc.gpsimd.collective_compute(
                    "ReduceScatter", ao.add,
                    replica_groups=[[0, 2, 4, 6], [1, 3, 5, 7]],
                    ins=[mixdram.ap().opt()],
                    outs=[rsdram.ap().opt()],
                )
            else:
                nc.sync.dma_start(rsdram[:], mixdram[0:CH])
            rs2 = rsdram[:].rearrange("(p f) -> p f", p=128)   # [128, 2816]
            o2 = out[:].rearrange("(p f) -> p f", p=128)
            for hh in range(2):
                oc = rvout.tile([128, FC], dt, tag="rv_out")
                nc.sync.dma_start(oc[:], rs2[:, FC * hh: FC * (hh + 1)])
                nc.vector.tensor_scalar(oc[:], oc[:], float(-CEIL), float(CEIL),
                                        op0=ao.max, op1=ao.min)
                nc.sync.dma_start(o2[:, FC * hh: FC * (hh + 1)], oc[:])

    nc.compile()
    return nc


# ============================================================== host wrapper
_CACHE = {}

def _get_program():
    if "nc" not in _CACHE:
        _CACHE["nc"] = build_program()
    return _CACHE["nc"]


def _host_consts():
    if "consts" in _CACHE:
        return _CACHE["consts"]
    ident = np.eye(128, dtype=np.float32)
    Lc = np.ascontiguousarray((0.25 * _epoch_matrix_comb(FB)).T.astype(np.float32))
    La = np.ascontiguousarray(_epoch_matrix_ap(FB).T.astype(np.float32))
    Laq = np.ascontiguousarray(_epoch_matrix_ap(FB, quirk=True).T.astype(np.float32))
    relpow = np.ascontiguousarray(np.broadcast_to(
        (REL ** (np.arange(FC, dtype=np.float64) + 1.0)).astype(np.float32),
        (128, FC)))
    eqc = {}
    for t in range(4):
        T, la_, lb_ = _track_eq_consts(t)
        eqc[t] = (np.ascontiguousarray(T.T.astype(np.float32)),
                  np.ascontiguousarray(la_.astype(np.float32)),
                  np.ascontiguousarray(lb_.astype(np.float32)))
    _CACHE["consts"] = (ident, Lc, La, Laq, relpow, eqc)
    return _CACHE["consts"]


def kernel(tracks, volumes, pans):
    tracks = np.ascontiguousarray(np.asarray(tracks, np.float32))
    volumes = np.asarray(volumes, np.float32)
    pans = np.asarray(pans, np.float32)

    angle = (pans.astype(np.float64) + 1.0) * 0.25 * math.pi
    lg, rg = np.cos(angle), np.sin(angle)
    ident, Lc, La, Laq, relpow, eqc = _host_consts()

    in_maps = []
    for core in range(8):
        t, ch = core // 2, core % 2
        xpad = np.zeros(NP, np.float32)
        xpad[:N] = tracks[t, ch]
        thT_np, ca_np, cb_np = eqc[t]
        w = float(volumes[t]) * float(lg[t] if ch == 0 else rg[t])
        has_rev = t >= 2
        w_dry = w * (1.0 - WET) if has_rev else w
        w_wet = w * WET if has_rev else 0.0
        in_maps.append({
            "x": xpad, "thT": thT_np, "ca": ca_np, "cb": cb_np,
            "ident": ident, "lcT": Lc, "laT": La, "laqT": Laq,
            "relpow": relpow,
            "wdry": np.full((128, 1), w_dry, np.float32),
            "wwet": np.full((128, 1), w_wet, np.float32),
        })

    nc = _get_program()
    res = run_bass_kernel_spmd(nc, in_maps, list(range(8)))

    outp = np.zeros((2, N), np.float32)
    for ch in range(2):
        full = np.concatenate([res.results[2 * q + ch]["out"] for q in range(4)])
        outp[ch] = full[:N]
    return outp
